# revision 37
# baseline (speedup 1.0000x reference)
import sys

import numpy as np

for _p in ("/opt/trn_rl_repo",):
    if _p not in sys.path:
        sys.path.insert(0, _p)

import ml_dtypes

import concourse.bass as bass
import concourse.bacc as bacc
import concourse.mybir as mybir
import concourse.tile as tile
from concourse.bass_utils import run_bass_kernel_spmd

F32 = mybir.dt.float32
BF16 = mybir.dt.bfloat16
AF = mybir.ActivationFunctionType
NPBF16 = ml_dtypes.bfloat16

# Problem dims (hardcoded per contract)
B, S, E, H = 2, 2048, 512, 32
D = E // H            # 16
NCORE = 8
HPC = H // NCORE      # 4 heads per core
C = HPC * D           # 64 channels per core
PADL = 127            # leading zeros in the flat softplus row
SPW = 2176            # padded flat row length (127 zeros + 2048 + 1 pad)
NT = S // 512         # 4 t-blocks of 512
NST = S // 128        # 16 s-tiles of 128

# bf16 blob column layout: wt (4*64) | wom0 (512) | wom1 (512) | id64 (64)
BW_WT, BW_W0, BW_W1, BW_ID = 0, 256, 768, 1280
BWCOLS = 1344
# f32 blob column layout: lts (128) | wrawT (64) | biasT (64) | inpb (1)
BF_LTS, BF_WR, BF_BT, BF_IB = 0, 128, 192, 256
BFCOLS = 257

_CACHE = {}


def _build_program():
    nc = bacc.Bacc()

    xT_d = nc.dram_tensor("xT", [E, B * S], BF16, kind="ExternalInput")
    inpwT_d = nc.dram_tensor("inpwT", [E, C], BF16, kind="ExternalInput")
    bfb_d = nc.dram_tensor("bfb", [128, BWCOLS], BF16, kind="ExternalInput")
    f32b_d = nc.dram_tensor("f32b", [128, BFCOLS], F32, kind="ExternalInput")
    sel4_d = nc.dram_tensor("sel4", [HPC, 128], BF16, kind="ExternalInput")
    out_d = nc.dram_tensor("out", [NT, 128, B * 4 * 512], BF16, kind="ExternalOutput")

    with tile.TileContext(nc) as tc:
        with (
            tc.tile_pool(name="persist", bufs=1) as pp,
            tc.tile_pool(name="wstage", bufs=2) as wstage,
            tc.tile_pool(name="dscr", bufs=1, space="DRAM") as dscr,
            tc.tile_pool(name="psh", bufs=2, space="PSUM") as psh,
            tc.tile_pool(name="pstp", bufs=2, space="PSUM") as pst_pool,
            tc.tile_pool(name="psy", bufs=2, space="PSUM") as psy,
            tc.tile_pool(name="pso", bufs=2, space="PSUM") as pso,
        ):
            # ---- constant loads: bfb heads the sync queue, f32b the scalar ----
            bfb = pp.tile([128, BWCOLS], BF16, tag="bfb")
            nc.sync.dma_start(bfb[:], bfb_d[:])
            f32b = pp.tile([128, BFCOLS], F32, tag="f32b")
            nc.scalar.dma_start(f32b[:], f32b_d[:])
            sel4 = pp.tile([HPC, 128], BF16, tag="sel4")
            nc.scalar.dma_start(sel4[:], sel4_d[:])
            wt = bfb[:, BW_WT : BW_WT + 256].rearrange("p (k c) -> p k c", k=4)
            wom = [
                bfb[:, BW_W0 : BW_W0 + 512],
                bfb[:, BW_W1 : BW_W1 + 512],
            ]
            id64 = bfb[0:64, BW_ID : BW_ID + 64]
            lts = f32b[:, BF_LTS : BF_LTS + 128]
            wrawT = f32b[:, BF_WR : BF_WR + C]
            biasT = f32b[:, BF_BT : BF_BT + C]
            inpb = f32b[0:C, BF_IB : BF_IB + 1]

            # ---- x tiles: 3 column chunks per slab; k0,k1 on sync queue,
            # k2,k3 on scalar. Chunk c0 heads both queues (gates proj q0).
            xk = [
                pp.tile([128, B * S], BF16, tag=f"xk{k}", name=f"xk{k}")
                for k in range(4)
            ]
            XCH = ((0, 1024), (1024, 2048), (2048, 4096))

            def load_x(c):
                lo, hi = XCH[c]
                for k in range(4):
                    eng = nc.sync if k < 2 else nc.scalar
                    eng.dma_start(
                        xk[k][:, lo:hi],
                        xT_d[k * 128 : (k + 1) * 128, lo:hi],
                    )

            load_x(0)

            # ---- softplus on compact layout, then Toeplitz-expand via DRAM ----
            # wsp[p, 16g+f] = softplus(w[head g][16p+f])  (fp32, feeds norm too)
            wsp = pp.tile([128, C], F32, tag="wsp")
            nc.scalar.activation(wsp[:], wrawT, AF.Exp)
            nc.scalar.activation(wsp[:], wsp[:], AF.Ln, bias=1.0)
            wsp16 = pp.tile([128, C], BF16, tag="wsp16")
            nc.vector.tensor_copy(wsp16[:], wsp[:])
            # flat rows in DRAM: spd[g, PADL + s] = softplus(w[g][s]); zeros before
            spd = dscr.tile([HPC, SPW], BF16, tag="spd")
            zt = pp.tile([HPC, PADL + 1], BF16, tag="zt")
            nc.gpsimd.memset(zt[:], 0.0)
            nc.gpsimd.dma_start(spd[:, 0:PADL], zt[:, 0:PADL])
            nc.gpsimd.dma_start(spd[:, PADL + S : SPW], zt[:, PADL : PADL + 1])
            for g in range(HPC):
                nc.gpsimd.dma_start(
                    spd[g : g + 1, PADL : PADL + S],
                    wsp16[:, g * 16 : (g + 1) * 16],
                )
            # expansion: mstar[g][p, j] = spd[g, p + j] = softplus(w[g][j - (127-p)])
            # (row p holds s-offset 127-p within a 128-tile; h_sb rows match)
            # cols 0:1024 via gpsimd SWDGE (needed first); cols 1024:2048 at the
            # tail of the scalar queue (mix consumes them last via descending si)
            mstar = [
                pp.tile([128, S], BF16, tag=f"mstar{g}", name=f"mstar{g}")
                for g in range(HPC)
            ]

            def expand(h, eng):
                for g in range(HPC):
                    src = spd[g : g + 1, h * 1024 : h * 1024 + 1024]
                    ap = src.copy()
                    ap.ap[0] = [1, 128]  # overlapping diagonal read
                    eng.dma_start(mstar[g][:, h * 1024 : (h + 1) * 1024], ap)

            expand(0, nc.gpsimd)

            # ---- norm = cumsum(softplus(w)) in layout s = 16p + f (fp32) ----
            cum = [wsp]
            for i, k in enumerate((1, 2, 4, 8)):
                nxt = pp.tile([128, C], F32, tag=f"cum{i}")
                prev = cum[-1]
                pv = prev[:].rearrange("p (g f) -> p g f", g=HPC)
                nv = nxt[:].rearrange("p (g f) -> p g f", g=HPC)
                nc.vector.tensor_add(
                    nv[:, :, k:16], pv[:, :, k:16], pv[:, :, 0 : 16 - k]
                )
                nc.vector.tensor_copy(nv[:, :, 0:k], pv[:, :, 0:k])
                cum.append(nxt)
            cfin = cum[-1]
            # chunk totals (p, g) at f=15, exclusive prefix over partitions via PE
            t128 = cfin[:].rearrange("p (g f) -> p g f", g=HPC)[:, :, 15]
            pe_e = psy.tile([128, 512], F32, tag="pyt", name="pe_e")
            nc.tensor.matmul(pe_e[:, 0:HPC], lts, t128, start=True, stop=True)
            norm = pp.tile([128, C], F32, tag="norm")
            nc.vector.tensor_add(
                norm[:].rearrange("p (g f) -> p g f", g=HPC),
                cfin[:].rearrange("p (g f) -> p g f", g=HPC),
                pe_e[:, 0:HPC, None].broadcast_to([128, HPC, 16]),
            )
            # compact [rnorm16 | bias*norm] bf16, then row-gather per head
            rbc = pp.tile([128, 2 * C], BF16, tag="rbc")
            rtmp = pp.tile([128, C], F32, tag="rtmp")
            nc.vector.reciprocal(rtmp[:], norm[:])
            nc.vector.tensor_copy(rbc[:, 0:C], rtmp[:])
            nc.vector.tensor_mul(rbc[:, C : 2 * C], biasT, norm[:])
            # rb gathers ride the sync queue right after x chunk c0
            rnr16 = pp.tile([HPC, S], BF16, tag="rnr16")
            bnr16 = pp.tile([HPC, S], BF16, tag="bnr16")
            for g in range(HPC):
                nc.sync.dma_start(
                    rnr16[g : g + 1, :], rbc[:, g * 16 : (g + 1) * 16]
                )
                nc.sync.dma_start(
                    bnr16[g : g + 1, :], rbc[:, C + g * 16 : C + (g + 1) * 16]
                )
            # second x half and late Toeplitz columns
            load_x(1)
            load_x(2)
            expand(1, nc.scalar)

            # ---- input projection: 2 concurrent col-tiles (b=0 / b=1) ----
            h_sb = [pp.tile([128, 128], BF16, tag=f"h{i}", name=f"h{i}") for i in range(NST)]
            y_sb = [pp.tile([128, 512], BF16, tag=f"y{j}", name=f"y{j}") for j in range(NT)]

            def inp_proj(q):
                hc = psh.tile([128, 512], F32, name="hc")
                for k in range(4):
                    for b in range(B):
                        nc.tensor.matmul(
                            hc[64 * b : 64 * b + 64, :],
                            wt[:, k, :],
                            xk[k][:, (2 * q + b) * 512 : (2 * q + b + 1) * 512],
                            start=(k == 0),
                            stop=(k == 3),
                            tile_position=(0, 64 * b),
                            skip_group_check=True,
                        )
                hcs = []
                for b in range(B):
                    hb = wstage.tile([64, 512], BF16, tag="hcs", bufs=4)
                    nc.vector.tensor_add(
                        hb[:],
                        hc[64 * b : 64 * b + 64, :],
                        inpb.broadcast_to([C, 512]),
                    )
                    hcs.append(hb)
                for tt in range(4):
                    st = q * 4 + tt
                    for b in range(B):
                        pst = pst_pool.tile([128, 64], BF16, name="pst")
                        nc.tensor.transpose(
                            pst[:],
                            hcs[b][:, tt * 128 : (tt + 1) * 128],
                            id64,
                        )
                        hv = h_sb[st][:].rearrange(
                            "p (g two d) -> p g two d", g=HPC, two=2
                        )[:, :, b, :]
                        nc.vector.tensor_copy(
                            hv, pst[:].rearrange("p (g d) -> p g d", g=HPC)
                        )

            def mix(tj):
                py = psy.tile([128, 512], F32, tag="pyt", name="py")
                # full blocks with ascending col offset first (defers the need
                # for mstar cols >=1024), then the diagonal partial blocks
                order = list(range(4 * tj, -1, -1)) + [
                    4 * tj + r for r in (1, 2, 3)
                ]
                for idx, si in enumerate(order):
                    dd = 512 * tj - 128 * si  # mstar col offset; <0 on diagonal
                    first = idx == 0
                    for g in range(HPC):
                        if dd >= 0:
                            nc.tensor.matmul(
                                py[32 * g : 32 * g + 32, :],
                                h_sb[si][:, 32 * g : 32 * g + 32],
                                mstar[g][:, dd : dd + 512],
                                start=first,
                                stop=False,
                                tile_position=(0, 32 * g),
                                skip_group_check=True,
                            )
                        else:
                            nc.tensor.matmul(
                                py[32 * g : 32 * g + 32, -dd : 512],
                                h_sb[si][:, 32 * g : 32 * g + 32],
                                mstar[g][:, 0 : 512 + dd],
                                start=False,
                                stop=False,
                                tile_position=(0, 32 * g),
                                skip_group_check=True,
                            )
                # + bias*norm broadcast to row groups (K=4 selector matmul)
                nc.tensor.matmul(
                    py[:],
                    sel4[:],
                    bnr16[0:HPC, 512 * tj : 512 * (tj + 1)],
                    start=False,
                    stop=True,
                    skip_group_check=True,
                )
                # rnorm broadcast to row groups, then y = psum * rnorm
                rm = psy.tile([128, 512], F32, tag="pyt", name="rm")
                nc.tensor.matmul(
                    rm[:],
                    sel4[:],
                    rnr16[0:HPC, 512 * tj : 512 * (tj + 1)],
                    start=True,
                    stop=True,
                )
                rms = wstage.tile([128, 512], F32, tag="rms", bufs=2)
                nc.scalar.activation(rms[:], rm[:], AF.Copy)
                nc.vector.tensor_mul(y_sb[tj][:], py[:], rms[:])

            def out_proj(tj):
                # obig cols: (b, tt, e); out_d[tj] is a straight 2-D copy
                obig = wstage.tile([128, 8 * 512], BF16, tag="obig", bufs=2)
                for b in range(B):
                    for tt4 in range(4):
                        po = pso.tile([128, E], F32, name="po")
                        nc.tensor.matmul(
                            po[:],
                            y_sb[tj][:, tt4 * 128 : tt4 * 128 + 128],
                            wom[b],
                            start=True,
                            stop=True,
                        )
                        dst = obig[:, (b * 4 + tt4) * 512 : (b * 4 + tt4 + 1) * 512]
                        if (tt4 * 2 + b) % 2 == 0:
                            nc.scalar.activation(dst, po[:], AF.Copy)
                        else:
                            nc.vector.tensor_copy(dst, po[:])
                    nc.gpsimd.dma_start(
                        out_d[tj][:, b * 2048 : (b + 1) * 2048],
                        obig[:, b * 2048 : (b + 1) * 2048],
                    )

            # emission order = expected data-ready order per engine queue.
            # mix(tj) needs only proj(<=tj); out(tj) needs mix(tj) + rmsf.
            inp_proj(0)
            mix(0)
            inp_proj(1)
            mix(1)
            out_proj(0)
            inp_proj(2)
            mix(2)
            out_proj(1)
            inp_proj(3)
            mix(3)
            out_proj(2)
            out_proj(3)
    nc.compile()
    return nc


def _host_prep(x, weight_raw, bias, inp_w, inp_b, out_w):
    x = np.asarray(x, np.float32)
    weight_raw = np.asarray(weight_raw, np.float32)
    bias = np.asarray(bias, np.float32)
    inp_w = np.asarray(inp_w, np.float32)
    inp_b = np.asarray(inp_b, np.float32)
    out_w = np.asarray(out_w, np.float32)

    xT = x.transpose(2, 0, 1).reshape(E, B, 4, 512)
    xT = xT.transpose(0, 2, 1, 3).reshape(E, B * S // 128, 128)
    # reverse s within each 128-tile: h_sb row p holds s = 128*si + 127 - p
    xT = np.ascontiguousarray(xT[:, :, ::-1].reshape(E, B * S)).astype(NPBF16)

    lts = np.tril(np.ones((128, 128), np.float32), -1).T.copy()
    sel4 = (np.arange(128)[None, :] // 32 == np.arange(HPC)[:, None]).astype(
        NPBF16
    )

    in_maps = []
    for core in range(NCORE):
        c0 = core * C
        heads = slice(core * HPC, (core + 1) * HPC)
        wrawT = (
            weight_raw[heads].reshape(HPC, 128, 16).transpose(1, 0, 2).reshape(128, C)
        )
        biasT = (
            bias[heads, :S].reshape(HPC, 128, 16).transpose(1, 0, 2).reshape(128, C)
        )
        f32b = np.zeros((128, BFCOLS), np.float32)
        f32b[:, BF_LTS : BF_LTS + 128] = lts
        f32b[:, BF_WR : BF_WR + C] = wrawT
        f32b[:, BF_BT : BF_BT + C] = biasT
        f32b[0:C, BF_IB] = inp_b[c0 : c0 + C]

        wo_slice = out_w[:, c0 : c0 + C].T.astype(np.float32)  # (C=g*16+d, E)
        bfb = np.zeros((128, BWCOLS), np.float32)
        bfb[:, BW_WT : BW_WT + 256] = (
            inp_w[c0 : c0 + C, :].T.reshape(4, 128, C).transpose(1, 0, 2).reshape(128, 256)
        )
        for b in range(B):
            v = bfb[:, BW_W0 + 512 * b : BW_W0 + 512 * (b + 1)].reshape(
                HPC, 2, D, E
            )
            v[:, b, :, :] = wo_slice.reshape(HPC, D, E)
        bfb[0:64, BW_ID : BW_ID + 64] = np.eye(64, dtype=np.float32)

        in_maps.append(
            {
                "xT": xT,
                "inpwT": np.ascontiguousarray(inp_w[c0 : c0 + C, :].T).astype(
                    NPBF16
                ),
                "bfb": bfb.astype(NPBF16),
                "f32b": f32b,
                "sel4": sel4,
            }
        )
    return in_maps


def _run(in_maps, trace=False):
    if "nc" not in _CACHE:
        _CACHE["nc"] = _build_program()
    try:
        res = run_bass_kernel_spmd(
            _CACHE["nc"], in_maps, core_ids=list(range(NCORE)), trace=trace
        )
    except ModuleNotFoundError:
        res = run_bass_kernel_spmd(
            _CACHE["nc"], in_maps, core_ids=list(range(NCORE)), trace=False
        )
    return res


def kernel(x, weight_raw, bias, inp_w, inp_b, out_w, parallel=True, _trace=False):
    in_maps = _host_prep(x, weight_raw, bias, inp_w, inp_b, out_w)
    res = _run(in_maps, trace=_trace)
    out = np.zeros((B, S, E), np.float32)
    for r in res.results:
        o = r["out"].astype(np.float32)  # [NT, 128, B*4*512]
        o = o.reshape(NT, 128, B, 4, 512).transpose(2, 0, 3, 1, 4)
        out += o.reshape(B, S, E)
    if _trace:
        kernel.last_exec_ns = res.exec_time_ns
        kernel.last_results = res
    return out


if __name__ == "__main__":
    rng = np.random.default_rng(0)
    inputs = {
        "x": rng.standard_normal((B, S, E), dtype=np.float32),
        "weight_raw": rng.standard_normal((H, S), dtype=np.float32),
        "bias": np.zeros((H, S), np.float32),
        "inp_w": rng.standard_normal((E, E), dtype=np.float32) / np.sqrt(E),
        "inp_b": np.zeros((E,), np.float32),
        "out_w": rng.standard_normal((E, E), dtype=np.float32) / np.sqrt(E),
    }
    o = kernel(**inputs)
    print("ok", o.shape, float(np.abs(o).mean()))


# revision 38
# speedup vs baseline: 1.0012x; 1.0012x over previous
import sys

import numpy as np

for _p in ("/opt/trn_rl_repo",):
    if _p not in sys.path:
        sys.path.insert(0, _p)

import ml_dtypes

import concourse.bass as bass
import concourse.bacc as bacc
import concourse.mybir as mybir
import concourse.tile as tile
from concourse.bass_utils import run_bass_kernel_spmd

F32 = mybir.dt.float32
BF16 = mybir.dt.bfloat16
AF = mybir.ActivationFunctionType
NPBF16 = ml_dtypes.bfloat16

# Problem dims (hardcoded per contract)
B, S, E, H = 2, 2048, 512, 32
D = E // H            # 16
NCORE = 8
HPC = H // NCORE      # 4 heads per core
C = HPC * D           # 64 channels per core
PADL = 127            # leading zeros in the flat softplus row
SPW = 2176            # padded flat row length (127 zeros + 2048 + 1 pad)
NT = S // 512         # 4 t-blocks of 512
NST = S // 128        # 16 s-tiles of 128

# bf16 blob column layout: wt (4*64) | wom0 (512) | wom1 (512) | id64 (64)
BW_WT, BW_W0, BW_W1, BW_ID = 0, 256, 768, 1280
BWCOLS = 1344
# f32 blob column layout: lts (128) | wrawT (64) | biasT (64) | inpb (1)
BF_LTS, BF_WR, BF_BT, BF_IB = 0, 128, 192, 256
BFCOLS = 257

_CACHE = {}


def _build_program():
    nc = bacc.Bacc()

    xT_d = nc.dram_tensor("xT", [E, B * S], BF16, kind="ExternalInput")
    inpwT_d = nc.dram_tensor("inpwT", [E, C], BF16, kind="ExternalInput")
    bfb_d = nc.dram_tensor("bfb", [128, BWCOLS], BF16, kind="ExternalInput")
    f32b_d = nc.dram_tensor("f32b", [128, BFCOLS], F32, kind="ExternalInput")
    sel4_d = nc.dram_tensor("sel4", [HPC, 128], BF16, kind="ExternalInput")
    out_d = nc.dram_tensor("out", [NT, 128, B * 4 * 512], BF16, kind="ExternalOutput")

    with tile.TileContext(nc) as tc:
        with (
            tc.tile_pool(name="persist", bufs=1) as pp,
            tc.tile_pool(name="wstage", bufs=2) as wstage,
            tc.tile_pool(name="dscr", bufs=1, space="DRAM") as dscr,
            tc.tile_pool(name="psh", bufs=2, space="PSUM") as psh,
            tc.tile_pool(name="pstp", bufs=2, space="PSUM") as pst_pool,
            tc.tile_pool(name="psy", bufs=2, space="PSUM") as psy,
            tc.tile_pool(name="pso", bufs=2, space="PSUM") as pso,
        ):
            # ---- constant loads: bfb heads the sync queue, f32b the scalar ----
            bfb = pp.tile([128, BWCOLS], BF16, tag="bfb")
            nc.sync.dma_start(bfb[:], bfb_d[:])
            f32b = pp.tile([128, BFCOLS], F32, tag="f32b")
            nc.scalar.dma_start(f32b[:], f32b_d[:])
            sel4 = pp.tile([HPC, 128], BF16, tag="sel4")
            nc.scalar.dma_start(sel4[:], sel4_d[:])
            wt = bfb[:, BW_WT : BW_WT + 256].rearrange("p (k c) -> p k c", k=4)
            wom = [
                bfb[:, BW_W0 : BW_W0 + 512],
                bfb[:, BW_W1 : BW_W1 + 512],
            ]
            id64 = bfb[0:64, BW_ID : BW_ID + 64]
            lts = f32b[:, BF_LTS : BF_LTS + 128]
            wrawT = f32b[:, BF_WR : BF_WR + C]
            biasT = f32b[:, BF_BT : BF_BT + C]
            inpb = f32b[0:C, BF_IB : BF_IB + 1]

            # ---- x tiles: 3 column chunks per slab; k0,k1 on sync queue,
            # k2,k3 on scalar. Chunk c0 heads both queues (gates proj q0).
            xk = [
                pp.tile([128, B * S], BF16, tag=f"xk{k}", name=f"xk{k}")
                for k in range(4)
            ]
            XCH = ((0, 1024), (1024, 2048), (2048, 4096))

            def load_x(c):
                lo, hi = XCH[c]
                for k in range(4):
                    eng = nc.sync if k < 2 else nc.scalar
                    eng.dma_start(
                        xk[k][:, lo:hi],
                        xT_d[k * 128 : (k + 1) * 128, lo:hi],
                    )

            load_x(0)

            # ---- softplus on compact layout, then Toeplitz-expand via DRAM ----
            # wsp[p, 16g+f] = softplus(w[head g][16p+f])  (fp32, feeds norm too)
            wsp = pp.tile([128, C], F32, tag="wsp")
            nc.scalar.activation(wsp[:], wrawT, AF.Exp)
            nc.scalar.activation(wsp[:], wsp[:], AF.Ln, bias=1.0)
            wsp16 = pp.tile([128, C], BF16, tag="wsp16")
            nc.vector.tensor_copy(wsp16[:], wsp[:])
            # flat rows in DRAM: spd[g, PADL + s] = softplus(w[g][s]); zeros before
            spd = dscr.tile([HPC, SPW], BF16, tag="spd")
            zt = pp.tile([HPC, PADL + 1], BF16, tag="zt")
            nc.gpsimd.memset(zt[:], 0.0)
            nc.gpsimd.dma_start(spd[:, 0:PADL], zt[:, 0:PADL])
            nc.gpsimd.dma_start(spd[:, PADL + S : SPW], zt[:, PADL : PADL + 1])
            for g in range(HPC):
                nc.gpsimd.dma_start(
                    spd[g : g + 1, PADL : PADL + S],
                    wsp16[:, g * 16 : (g + 1) * 16],
                )
            # expansion: mstar[g][p, j] = spd[g, p + j] = softplus(w[g][j - (127-p)])
            # (row p holds s-offset 127-p within a 128-tile; h_sb rows match)
            # cols 0:1024 via gpsimd SWDGE (needed first); cols 1024:2048 at the
            # tail of the scalar queue (mix consumes them last via descending si)
            mstar = [
                pp.tile([128, S], BF16, tag=f"mstar{g}", name=f"mstar{g}")
                for g in range(HPC)
            ]

            def expand(h, eng):
                for g in range(HPC):
                    src = spd[g : g + 1, h * 1024 : h * 1024 + 1024]
                    ap = src.copy()
                    ap.ap[0] = [1, 128]  # overlapping diagonal read
                    eng.dma_start(mstar[g][:, h * 1024 : (h + 1) * 1024], ap)

            expand(0, nc.gpsimd)

            # ---- norm = cumsum(softplus(w)) in layout s = 16p + f (fp32) ----
            cum = [wsp]
            for i, k in enumerate((1, 2, 4, 8)):
                nxt = pp.tile([128, C], F32, tag=f"cum{i}")
                prev = cum[-1]
                pv = prev[:].rearrange("p (g f) -> p g f", g=HPC)
                nv = nxt[:].rearrange("p (g f) -> p g f", g=HPC)
                nc.vector.tensor_add(
                    nv[:, :, k:16], pv[:, :, k:16], pv[:, :, 0 : 16 - k]
                )
                nc.vector.tensor_copy(nv[:, :, 0:k], pv[:, :, 0:k])
                cum.append(nxt)
            cfin = cum[-1]
            # chunk totals (p, g) at f=15, exclusive prefix over partitions via PE
            t128 = cfin[:].rearrange("p (g f) -> p g f", g=HPC)[:, :, 15]
            pe_e = psy.tile([128, 512], F32, tag="pyt", name="pe_e")
            nc.tensor.matmul(pe_e[:, 0:HPC], lts, t128, start=True, stop=True)
            norm = pp.tile([128, C], F32, tag="norm")
            nc.vector.tensor_add(
                norm[:].rearrange("p (g f) -> p g f", g=HPC),
                cfin[:].rearrange("p (g f) -> p g f", g=HPC),
                pe_e[:, 0:HPC, None].broadcast_to([128, HPC, 16]),
            )
            # compact [rnorm16 | bias*norm] bf16, then row-gather per head
            rbc = pp.tile([128, 2 * C], BF16, tag="rbc")
            rtmp = pp.tile([128, C], F32, tag="rtmp")
            nc.vector.reciprocal(rtmp[:], norm[:])
            nc.vector.tensor_copy(rbc[:, 0:C], rtmp[:])
            nc.vector.tensor_mul(rbc[:, C : 2 * C], biasT, norm[:])
            # rb gathers ride the sync queue right after x chunk c0
            rnr16 = pp.tile([HPC, S], BF16, tag="rnr16")
            bnr16 = pp.tile([HPC, S], BF16, tag="bnr16")
            for g in range(HPC):
                nc.sync.dma_start(
                    rnr16[g : g + 1, :], rbc[:, g * 16 : (g + 1) * 16]
                )
                nc.sync.dma_start(
                    bnr16[g : g + 1, :], rbc[:, C + g * 16 : C + (g + 1) * 16]
                )
            # second x half and late Toeplitz columns
            load_x(1)
            load_x(2)
            expand(1, nc.scalar)

            # ---- input projection: 2 concurrent col-tiles (b=0 / b=1) ----
            h_sb = [pp.tile([128, 128], BF16, tag=f"h{i}", name=f"h{i}") for i in range(NST)]
            y_sb = [pp.tile([128, 512], BF16, tag=f"y{j}", name=f"y{j}") for j in range(NT)]

            def inp_proj(q):
                hc = psh.tile([128, 512], F32, name="hc")
                for k in range(4):
                    for b in range(B):
                        nc.tensor.matmul(
                            hc[64 * b : 64 * b + 64, :],
                            wt[:, k, :],
                            xk[k][:, (2 * q + b) * 512 : (2 * q + b + 1) * 512],
                            start=(k == 0),
                            stop=(k == 3),
                            tile_position=(0, 64 * b),
                            skip_group_check=True,
                        )
                hcs = []
                for b in range(B):
                    hb = wstage.tile([64, 512], BF16, tag="hcs", bufs=4)
                    nc.vector.tensor_add(
                        hb[:],
                        hc[64 * b : 64 * b + 64, :],
                        inpb.broadcast_to([C, 512]),
                    )
                    hcs.append(hb)
                for tt in range(4):
                    st = q * 4 + tt
                    for b in range(B):
                        pst = pst_pool.tile([128, 64], BF16, name="pst")
                        nc.tensor.transpose(
                            pst[:],
                            hcs[b][:, tt * 128 : (tt + 1) * 128],
                            id64,
                        )
                        hv = h_sb[st][:].rearrange(
                            "p (g two d) -> p g two d", g=HPC, two=2
                        )[:, :, b, :]
                        nc.vector.tensor_copy(
                            hv, pst[:].rearrange("p (g d) -> p g d", g=HPC)
                        )

            def mix(tj):
                py = psy.tile([128, 512], F32, tag="pyt", name="py")
                # full blocks with ascending col offset first (defers the need
                # for mstar cols >=1024), then the diagonal partial blocks
                order = list(range(4 * tj, -1, -1)) + [
                    4 * tj + r for r in (1, 2, 3)
                ]
                for idx, si in enumerate(order):
                    dd = 512 * tj - 128 * si  # mstar col offset; <0 on diagonal
                    first = idx == 0
                    for g in range(HPC):
                        if dd >= 0:
                            nc.tensor.matmul(
                                py[32 * g : 32 * g + 32, :],
                                h_sb[si][:, 32 * g : 32 * g + 32],
                                mstar[g][:, dd : dd + 512],
                                start=first,
                                stop=False,
                                tile_position=(0, 32 * g),
                                skip_group_check=True,
                            )
                        else:
                            nc.tensor.matmul(
                                py[32 * g : 32 * g + 32, -dd : 512],
                                h_sb[si][:, 32 * g : 32 * g + 32],
                                mstar[g][:, 0 : 512 + dd],
                                start=False,
                                stop=False,
                                tile_position=(0, 32 * g),
                                skip_group_check=True,
                            )
                # + bias*norm broadcast to row groups (K=4 selector matmul)
                nc.tensor.matmul(
                    py[:],
                    sel4[:],
                    bnr16[0:HPC, 512 * tj : 512 * (tj + 1)],
                    start=False,
                    stop=True,
                    skip_group_check=True,
                )
                # rnorm broadcast to row groups, then y = psum * rnorm
                rm = psy.tile([128, 512], F32, tag="pyt", name="rm")
                nc.tensor.matmul(
                    rm[:],
                    sel4[:],
                    rnr16[0:HPC, 512 * tj : 512 * (tj + 1)],
                    start=True,
                    stop=True,
                )
                rms = wstage.tile([128, 512], F32, tag="rms", bufs=2)
                nc.scalar.activation(rms[:], rm[:], AF.Copy)
                nc.vector.tensor_mul(y_sb[tj][:], py[:], rms[:])

            def out_proj(tj):
                # obig cols: (b, tt, e); out_d[tj] is a straight 2-D copy
                obig = wstage.tile([128, 8 * 512], BF16, tag="obig", bufs=2)
                for b in range(B):
                    for tt4 in range(4):
                        po = pso.tile([128, E], F32, name="po")
                        nc.tensor.matmul(
                            po[:],
                            y_sb[tj][:, tt4 * 128 : tt4 * 128 + 128],
                            wom[b],
                            start=True,
                            stop=True,
                        )
                        dst = obig[:, (b * 4 + tt4) * 512 : (b * 4 + tt4 + 1) * 512]
                        if (tt4 * 2 + b) % 2 == 0:
                            nc.scalar.activation(dst, po[:], AF.Copy)
                        else:
                            nc.vector.tensor_copy(dst, po[:])
                    # spread writes: early blocks on the SWDGE queue, the tail
                    # blocks on the by-then-idle HWDGE queues
                    oeng = nc.gpsimd if tj < 2 else (nc.sync if tj == 2 else nc.scalar)
                    oeng.dma_start(
                        out_d[tj][:, b * 2048 : (b + 1) * 2048],
                        obig[:, b * 2048 : (b + 1) * 2048],
                    )

            # emission order = expected data-ready order per engine queue.
            # mix(tj) needs only proj(<=tj); out(tj) needs mix(tj) + rmsf.
            inp_proj(0)
            mix(0)
            inp_proj(1)
            mix(1)
            out_proj(0)
            inp_proj(2)
            mix(2)
            out_proj(1)
            inp_proj(3)
            mix(3)
            out_proj(2)
            out_proj(3)
    nc.compile()
    return nc


def _host_prep(x, weight_raw, bias, inp_w, inp_b, out_w):
    x = np.asarray(x, np.float32)
    weight_raw = np.asarray(weight_raw, np.float32)
    bias = np.asarray(bias, np.float32)
    inp_w = np.asarray(inp_w, np.float32)
    inp_b = np.asarray(inp_b, np.float32)
    out_w = np.asarray(out_w, np.float32)

    xT = x.transpose(2, 0, 1).reshape(E, B, 4, 512)
    xT = xT.transpose(0, 2, 1, 3).reshape(E, B * S // 128, 128)
    # reverse s within each 128-tile: h_sb row p holds s = 128*si + 127 - p
    xT = np.ascontiguousarray(xT[:, :, ::-1].reshape(E, B * S)).astype(NPBF16)

    lts = np.tril(np.ones((128, 128), np.float32), -1).T.copy()
    sel4 = (np.arange(128)[None, :] // 32 == np.arange(HPC)[:, None]).astype(
        NPBF16
    )

    in_maps = []
    for core in range(NCORE):
        c0 = core * C
        heads = slice(core * HPC, (core + 1) * HPC)
        wrawT = (
            weight_raw[heads].reshape(HPC, 128, 16).transpose(1, 0, 2).reshape(128, C)
        )
        biasT = (
            bias[heads, :S].reshape(HPC, 128, 16).transpose(1, 0, 2).reshape(128, C)
        )
        f32b = np.zeros((128, BFCOLS), np.float32)
        f32b[:, BF_LTS : BF_LTS + 128] = lts
        f32b[:, BF_WR : BF_WR + C] = wrawT
        f32b[:, BF_BT : BF_BT + C] = biasT
        f32b[0:C, BF_IB] = inp_b[c0 : c0 + C]

        wo_slice = out_w[:, c0 : c0 + C].T.astype(np.float32)  # (C=g*16+d, E)
        bfb = np.zeros((128, BWCOLS), np.float32)
        bfb[:, BW_WT : BW_WT + 256] = (
            inp_w[c0 : c0 + C, :].T.reshape(4, 128, C).transpose(1, 0, 2).reshape(128, 256)
        )
        for b in range(B):
            v = bfb[:, BW_W0 + 512 * b : BW_W0 + 512 * (b + 1)].reshape(
                HPC, 2, D, E
            )
            v[:, b, :, :] = wo_slice.reshape(HPC, D, E)
        bfb[0:64, BW_ID : BW_ID + 64] = np.eye(64, dtype=np.float32)

        in_maps.append(
            {
                "xT": xT,
                "inpwT": np.ascontiguousarray(inp_w[c0 : c0 + C, :].T).astype(
                    NPBF16
                ),
                "bfb": bfb.astype(NPBF16),
                "f32b": f32b,
                "sel4": sel4,
            }
        )
    return in_maps


def _run(in_maps, trace=False):
    if "nc" not in _CACHE:
        _CACHE["nc"] = _build_program()
    try:
        res = run_bass_kernel_spmd(
            _CACHE["nc"], in_maps, core_ids=list(range(NCORE)), trace=trace
        )
    except ModuleNotFoundError:
        res = run_bass_kernel_spmd(
            _CACHE["nc"], in_maps, core_ids=list(range(NCORE)), trace=False
        )
    return res


def kernel(x, weight_raw, bias, inp_w, inp_b, out_w, parallel=True, _trace=False):
    in_maps = _host_prep(x, weight_raw, bias, inp_w, inp_b, out_w)
    res = _run(in_maps, trace=_trace)
    out = np.zeros((B, S, E), np.float32)
    for r in res.results:
        o = r["out"].astype(np.float32)  # [NT, 128, B*4*512]
        o = o.reshape(NT, 128, B, 4, 512).transpose(2, 0, 3, 1, 4)
        out += o.reshape(B, S, E)
    if _trace:
        kernel.last_exec_ns = res.exec_time_ns
        kernel.last_results = res
    return out


if __name__ == "__main__":
    rng = np.random.default_rng(0)
    inputs = {
        "x": rng.standard_normal((B, S, E), dtype=np.float32),
        "weight_raw": rng.standard_normal((H, S), dtype=np.float32),
        "bias": np.zeros((H, S), np.float32),
        "inp_w": rng.standard_normal((E, E), dtype=np.float32) / np.sqrt(E),
        "inp_b": np.zeros((E,), np.float32),
        "out_w": rng.standard_normal((E, E), dtype=np.float32) / np.sqrt(E),
    }
    o = kernel(**inputs)
    print("ok", o.shape, float(np.abs(o).mean()))
